# revision 21
# baseline (speedup 1.0000x reference)
"""AntiPatternLoss Trainium2 kernel (8 NeuronCores, data-parallel over batch).

Reference computation (per batch row of logits [T=2048, V=128]):
  pred      = argmax_v(logits)                                    # [T]
  prob_pred = softmax(logits)[t, pred[t]] = maxexp / sum_v exp(l)
  pen[j]    = mean_{k<3} prob_pred[j+k]                           # [L], L = T-2
  eq[i,j]   = (trigram at i == trigram at j) and (j - i >= 3)
  loss      = REP_PEN * sum_j(count_j * pen_j) / (B*T)

Kernel strategy per core (2 rows):
  - key tiles = exp(l) f32 bits (positive IEEE bits are order-isomorphic
    to the value).  argmax per position = i32 max-reduce of RAW bits
    (= exact f32 max), then the index is recovered with is_eq against
    the broadcast max + multiply by (127-v) + a second max-reduce
    (ties pick the smallest v, matching jnp.argmax).  The max bits
    double as maxexp, and sum-reduce of the same tiles is sumexp, so
    softmax needs no extra exp pass and no bit masking at all.
  - trigram code = p0*16384 + p1*128 + p2 (< 2^21, exact in fp32);
    pairwise match is ONE fp32 equality compare (2x DVE mode).
  - near-diagonal blocks for ALL 16 i-tiles of a row are ONE TT is_eq
    (code_ipart broadcast along the free tile dim with stride 0), then
    one staircase mask multiply; their count matmuls carry start=True
    per PSUM bank (no zero-pass), edge cols are memset directly.
  - far tiles split 3 ways: t in POOL_T on GpSimd (1-pass is_eq),
    t < SPLIT_T on DVE (1-pass is_eq), the rest on ScalarE (Abs+Relu,
    2-pass); the i-reduction runs on TensorE as ones-matmul
    accumulating counts[1, j] in PSUM.
  - constant tables (wrev, staircase mask, identity, ones, sentinels)
    are packed host-side into one f32 input tensor and DMA'd once.
  - per-core partial loss scalars are summed on the host (gather step)
"""

import numpy as np

import concourse.mybir as mybir
from concourse import bacc, tile
from concourse.bass_utils import run_bass_kernel_spmd

F32 = mybir.dt.float32
BF16 = mybir.dt.bfloat16
I32 = mybir.dt.int32
AL = mybir.AluOpType
AF = mybir.ActivationFunctionType

N_CORES = 8
B, T, V = 16, 2048, 128
R = B // N_CORES          # rows per core = 2
NGRAM = 3
REP_PEN = 1.2
L = T - NGRAM + 1         # 2046 trigram start positions
NT = T // 128             # 16 i-tiles per row
PAD = 2                   # sentinel cols in front of codes in code_bcast
SENT_BC = -1.0            # j-side (code_bcast / code flat) sentinel
SENT_I = -3.0             # i-side (code_ipart) sentinel
SCALE = REP_PEN / (NGRAM * B * T)   # pen's /3 folded in
CB_W = PAD + T + 8        # code_bcast width
NCH = 4
HALF = NT // NCH

# far-tile engine split: per i-tile t (0..14), W_t = 1915 - 128t
DVE_T = (0, 1, 2, 3, 4, 5, 6, 7)  # 1-pass is_eq on DVE
POOL_T = ()                   # GpSimd is_eq is ~15 ns/col ucode - unusable
# remainder (7..14) on ScalarE via Abs+Relu (2-pass); those only need
# the first (tail) half of the code broadcast, so they start earliest

# ---- host-packed constant table (one [128, CONST_W] f32 DMA) -----------
C_DIAG = 0            # [128,1024] f32-packed bf16 [128,2048] staircase mask
C_ID16 = 1024         # [16,8] f32-packed bf16 [16,16] identity
C_ISH = 1032          # [128,128] f32 ishift (subdiagonal identity)
C_ONEF = 1160         # [128,1] f32 ones
C_ONEB = 1161         # [128,1] f32-packed bf16 [128,2] ones
C_SENTI = 1162        # [128,1] f32: rows 0-1 = SENT_I
C_SENTB = 1163        # [1,4] f32 row 0 = SENT_BC
C_WREVB = 1167        # [128,1024] f32-packed bf16 [128,2048] wrev rep 16x
CONST_W = 2191


def _pack_bf16(a32: np.ndarray) -> np.ndarray:
    """f32 array [P, W] (W even) -> bf16-bit-packed f32 array [P, W//2]."""
    bf = (np.ascontiguousarray(a32, np.float32).view(np.uint32) >> 16) \
        .astype(np.uint16)
    return np.ascontiguousarray(bf).view(np.float32)


def _build_consts() -> np.ndarray:
    c = np.zeros((128, CONST_W), dtype=np.float32)
    p = np.arange(128)[:, None]
    cc = np.arange(128)[None, :]
    # diag staircase mask, same for every i-tile: keep col j-offset c
    # (0..127) iff c >= p  (j - i = 3 + c - p >= 3)
    stair = (cc >= p).astype(np.float32)                   # [128,128]
    c[:, C_DIAG:C_DIAG + 1024] = _pack_bf16(np.tile(stair, (1, 16)))
    c[0:16, C_ID16:C_ID16 + 8] = _pack_bf16(np.eye(16, dtype=np.float32))
    # ishift f32 [128,128]: ishift[p, c] = 1 if c == p - 1 else 0
    c[:, C_ISH:C_ISH + 128] = (cc == (p - 1)).astype(np.float32)
    c[:, C_ONEF] = 1.0
    c[:, C_ONEB:C_ONEB + 1] = _pack_bf16(np.ones((128, 2), dtype=np.float32))
    c[0:2, C_SENTI] = SENT_I
    c[0, C_SENTB:C_SENTB + 4] = SENT_BC
    wrev = np.tile((127.0 - np.arange(128, dtype=np.float32))[None, :],
                   (128, 16))
    c[:, C_WREVB:C_WREVB + 1024] = _pack_bf16(wrev)
    return np.ascontiguousarray(c)


def _bank_chunks(a, b):
    """Split [a, b) at 512-column PSUM bank boundaries."""
    out = []
    while a < b:
        nxt = min(b, (a // 512 + 1) * 512)
        out.append((a, nxt))
        a = nxt
    return out


def build_nc():
    nc = bacc.Bacc("TRN2", target_bir_lowering=False, debug=False,
                   num_devices=N_CORES)
    x_ext = nc.dram_tensor("logits", [R * T, V], F32, kind="ExternalInput")
    c_ext = nc.dram_tensor("consts", [128, CONST_W], F32, kind="ExternalInput")
    y_ext = nc.dram_tensor("out", [1, 1], F32, kind="ExternalOutput")

    with tile.TileContext(nc) as tc:
        with (
            tc.tile_pool(name="setup", bufs=1) as setup,
            tc.tile_pool(name="big", bufs=1) as big,
            tc.tile_pool(name="small", bufs=1) as small,
            tc.tile_pool(name="eqp", bufs=8) as eqp,
            tc.tile_pool(name="scr", bufs=1) as scrp,
            tc.tile_pool(name="ps", bufs=1, space="PSUM") as ps,
            tc.tile_pool(name="dram", bufs=1, space="DRAM") as dram,
        ):
            # ---------------- constants: one DMA on the scalar queue ------
            consts = setup.tile([128, CONST_W], F32)
            nc.gpsimd.dma_start(consts[:], c_ext.ap())
            diagmask = consts[:, C_DIAG:C_DIAG + 1024].bitcast(BF16)  # [128,2048]
            ident16 = consts[0:16, C_ID16:C_ID16 + 8].bitcast(BF16)   # [16,16]
            ishift = consts[:, C_ISH:C_ISH + 128]
            ones_f32 = consts[:, C_ONEF:C_ONEF + 1]
            ones_bf = consts[:, C_ONEB:C_ONEB + 1].bitcast(BF16)[:, 0:1]
            sentI = consts[0:2, C_SENTI:C_SENTI + 1]
            sentBC = consts[0:1, C_SENTB:C_SENTB + 4]
            wrevb = consts[:, C_WREVB:C_WREVB + 1024].bitcast(BF16)   # [128,2048]

            counts_ps = ps.tile([32 * (R - 1) + 1, T], F32)
            ps_fin = ps.tile([1, R], F32)
            s1c = small.tile([128, R], F32)
            junkR = scrp.tile([1, R], F32)
            final_sb = scrp.tile([1, 1], F32)

            x = x_ext.ap()
            dmae = nc.sync

            # ====== per-row pipeline: row0 entirely first, then row1 ======
            # row0's first chunk halved so its key chain starts earlier
            CHUNKS = {0: [(0, 2), (2, 4), (4, 8), (8, 12), (12, 16)],
                      1: [(0, 4), (4, 8), (8, 12), (12, 16)]}
            key_all = {}
            rk_all = {}
            for r in range(R):
                for h, (b0, b1) in enumerate(CHUNKS[r]):
                    lgh = big.tile([128, HALF * 128], F32, tag="lg", bufs=9,
                                   name=f"logits_sb{r}{h}")
                    key_all[(r, h)] = lgh
                    src = x[r * T:(r + 1) * T, :] \
                        .rearrange("(a b) v -> a (b v)", a=128)[:, b0 * 128 * V // 128:b1 * V]
                    dmae.dma_start(lgh[:, 0:(b1 - b0) * 128], src)
            for r in range(R):
                red_key = small.tile([128, NT], I32, name=f"rk_{r}")
                idxw = small.tile([128, NT], BF16, name=f"idxw_{r}")
                rk_all[r] = red_key
                for h, (b0, b1) in enumerate(CHUNKS[r]):
                    w = (b1 - b0) * 128
                    nb = b1 - b0
                    lgh = key_all[(r, h)][:, 0:w]
                    key_t = big.tile([128, HALF * 128], I32, tag="ky", bufs=9,
                                     name=f"key_{r}{h}")
                    key_all[(r, h)] = key_t   # retained: doubles as exp(l)
                    key = key_t[:, 0:w]
                    # key = exp(l): raw positive f32 bits are a monotone
                    # i32 key, and the tile doubles as E0 for softmax
                    nc.scalar.activation(key.bitcast(F32), lgh, AF.Exp,
                                         bias=0.0, scale=1.0)
                    k3 = key.rearrange("p (b v) -> p b v", v=128)
                    bs = slice(b0, b1)
                    with nc.allow_low_precision(reason="int32 max reduce"):
                        nc.vector.tensor_reduce(
                            out=red_key[:, bs], in_=k3,
                            axis=mybir.AxisListType.X, op=AL.max)
                    # recover first-max index: (key == max) * (127-v), max
                    rkb = red_key[:, bs].rearrange("p (b o) -> p b o", o=1) \
                        .broadcast_to([128, nb, 128])
                    eqm_t = big.tile([128, HALF * 128], BF16, tag="eqm",
                                     bufs=3, name=f"eqm_{r}{h}")
                    eqm = eqm_t[:, 0:w].rearrange("p (b v) -> p b v", v=128)
                    nc.vector.tensor_tensor(out=eqm, in0=k3, in1=rkb,
                                            op=AL.is_equal)
                    wb = wrevb[:, 0:w].rearrange("p (b v) -> p b v", v=128)
                    nc.vector.tensor_tensor(out=eqm, in0=eqm, in1=wb,
                                            op=AL.mult)
                    nc.vector.tensor_reduce(
                        out=idxw[:, bs], in_=eqm,
                        axis=mybir.AxisListType.X, op=AL.max)

                # decode pred as bf16 (values <= 127, exact)
                pred_bf = small.tile([128, NT], BF16, name=f"pred_bf{r}")
                nc.vector.tensor_scalar(out=pred_bf[:], in0=idxw[:],
                                        scalar1=-1.0, scalar2=127.0,
                                        op0=AL.mult, op1=AL.add)

                # next-partition pred values via a tiny DMA shift
                pshift = small.tile([128, 2], BF16, name=f"pshift{r}")
                nc.gpsimd.memset(pshift[:], 0.0)  # junk fill (row 127 unused)
                nc.gpsimd.dma_start(pshift[0:127, :], pred_bf[1:128, 0:2])

                # pred -> SBUF flat [16, 130] -> PE transposes
                flat16 = small.tile([16, 130], BF16, name=f"flat16_{r}")
                nc.vector.memset(flat16[:], 0.0)
                nc.gpsimd.dma_start(flat16[0:16, 0:128], pred_bf[:])
                # wrap columns: flat[128q + 128 + c] = pred_bf[8(q+1), c]
                nc.gpsimd.dma_start(
                    flat16[0:15, 128:130],
                    pred_bf[:].rearrange("(q e) b -> q e b", e=8)[1:16, 0, 0:2])
                # code_ipart[p, t] = code[128t + p] via 3 PE transposes
                tp_ps = ps.tile([128, 3 * NT], BF16, tag="tp", name=f"tp_ps{r}")
                for k in range(3):
                    nc.tensor.transpose(tp_ps[:, NT * k:NT * (k + 1)],
                                        flat16[0:16, k:k + 128], ident16)
                p0t = tp_ps[:, 0:NT]
                p1t = tp_ps[:, NT:2 * NT]
                p2t = tp_ps[:, 2 * NT:3 * NT]
                ipt_a = small.tile([128, NT], F32, name=f"ipt_a{r}")
                ipt_b = small.tile([128, NT], F32, name=f"ipt_b{r}")
                code_ipart = small.tile([128, NT], F32, name=f"code_ipart{r}")
                nc.vector.tensor_scalar(out=ipt_a[:], in0=p0t, scalar1=16384.0,
                                        scalar2=None, op0=AL.mult)
                nc.vector.scalar_tensor_tensor(out=ipt_b[:], in0=p1t, scalar=128.0,
                                               in1=ipt_a[:], op0=AL.mult, op1=AL.add)
                nc.vector.tensor_tensor(out=code_ipart[:], in0=ipt_b[:], in1=p2t,
                                        op=AL.add)
                nc.gpsimd.dma_start(code_ipart[126:128, NT - 1:NT], sentI[:])
                negip = small.tile([128, NT], F32, name=f"negip{r}")
                nc.gpsimd.tensor_scalar(out=negip[:], in0=code_ipart[:],
                                        scalar1=-1.0, scalar2=None, op0=AL.mult)

                # trigram codes: code = p0*16384 + p1*128 + p2 via fused
                # DVE stt ops on shifted views of pred_bf (+pshift patches)
                tmp_b = small.tile([128, NT], F32, name=f"tmp_b{r}")
                code2 = small.tile([128, NT], F32, name=f"code2{r}")
                nc.vector.scalar_tensor_tensor(
                    out=tmp_b[:, 0:NT - 2], in0=pred_bf[:, 1:NT - 1],
                    scalar=128.0, in1=pred_bf[:, 2:NT],
                    op0=AL.mult, op1=AL.add)
                nc.vector.scalar_tensor_tensor(
                    out=tmp_b[:, NT - 2:NT - 1], in0=pred_bf[:, NT - 1:NT],
                    scalar=128.0, in1=pshift[:, 0:1],
                    op0=AL.mult, op1=AL.add)
                nc.vector.scalar_tensor_tensor(
                    out=tmp_b[:, NT - 1:NT], in0=pshift[:, 0:1],
                    scalar=128.0, in1=pshift[:, 1:2],
                    op0=AL.mult, op1=AL.add)
                nc.vector.scalar_tensor_tensor(
                    out=code2[:], in0=pred_bf[:],
                    scalar=16384.0, in1=tmp_b[:],
                    op0=AL.mult, op1=AL.add)

                # -------- distribute codes ------------------------------
                # stage via two independent DRAM tiles (tail first) and
                # broadcast back; positions 2046/2047 (invalid trigram
                # starts) get sentineled in cb directly after the bcast
                cft = dram.tile([1, 1024], F32, name=f"cft{r}")
                cfh = dram.tile([1, 1024], F32, name=f"cfh{r}")
                cft128 = cft[:].rearrange("o (a b) -> (o a) b", a=64)
                cfh128 = cfh[:].rearrange("o (a b) -> (o a) b", a=64)
                nc.gpsimd.dma_start(cft128[:, :], code2[64:128, :])
                nc.gpsimd.dma_start(cfh128[:, :], code2[0:64, :])

                cb = big.tile([128, CB_W], F32, tag=f"cb{r}", name=f"code_bcast{r}")
                nc.gpsimd.memset(cb[:, 0:PAD], SENT_BC)
                dmae.dma_start(cb[:, PAD + 1024:PAD + T],
                               cft[:].partition_broadcast(128))
                nc.gpsimd.memset(cb[:, PAD + L:CB_W], SENT_BC)
                nc.gpsimd.dma_start(cb[:, PAD:PAD + 1024],
                               cfh[:].partition_broadcast(128))

                # -------- pairwise match counting ------------------------
                # PSUM edge cols (j < 3, j >= L) never get a matmul: zero
                # them directly
                nc.vector.memset(counts_ps[32 * r:32 * r + 1, 0:3], 0.0)
                nc.vector.memset(counts_ps[32 * r:32 * r + 1, L:T], 0.0)

                # ScalarE far tiles first: they only need the tail (first-
                # landing) half of cb, so they start earliest
                sc_tiles = [t for t in range(NT - 1)
                            if t not in DVE_T and t not in POOL_T]
                eq_sc = {}
                for t in sc_tiles:
                    W = L - (128 * t + 131)
                    if W <= 0:
                        continue
                    eqt = eqp.tile([128, 1152], BF16, tag="eqt", bufs=9,
                                   name=f"eqt{r}_{t}")
                    at = eqp.tile([128, 1152], BF16, tag="at", bufs=9,
                                  name=f"at{r}_{t}")
                    nc.scalar.activation(at[:, 0:W],
                                         cb[:, PAD + 128 * t + 131:PAD + L],
                                         AF.Abs, bias=negip[:, t:t + 1],
                                         scale=1.0)
                    nc.scalar.activation(eqt[:, 0:W], at[:, 0:W],
                                         AF.Relu, bias=1.0, scale=-1.0)
                    eq_sc[t] = eqt

                # near-diagonal blocks for all 16 tiles in ONE TT is_eq
                # (code_ipart broadcast along the c dim), then the
                # staircase mask; these matmuls open each PSUM bank
                eqd = big.tile([128, NT * 128], BF16, tag=f"eqd{r}", name=f"eqd{r}")
                cb3 = cb[:, PAD + 3:PAD + 3 + T].rearrange(
                    "p (t c) -> p t c", c=128)
                e3 = eqd[:].rearrange("p (t c) -> p t c", c=128)
                cip_b = code_ipart[:].rearrange("p (t o) -> p t o", o=1) \
                    .broadcast_to([128, NT, 128])
                nc.vector.tensor_tensor(out=e3, in0=cb3, in1=cip_b,
                                        op=AL.is_equal)
                nc.vector.tensor_tensor(out=eqd[:, 0:T], in0=eqd[:, 0:T],
                                        in1=diagmask[:, 0:T], op=AL.mult)
                for (a, b2) in _bank_chunks(3, L):
                    nc.tensor.matmul(
                        counts_ps[32 * r:32 * r + 1, a:b2], ones_bf,
                        eqd[:, a - 3:b2 - 3],
                        start=True, stop=True, skip_group_check=True)

                # far tiles: DVE + GpSimd (1-pass is_eq each)
                for t in POOL_T + DVE_T:
                    W = L - (128 * t + 131)
                    if W <= 0:
                        continue
                    eqt = eqp.tile([128, 1920], BF16, tag="eqv", bufs=4,
                                   name=f"eqv{r}_{t}")
                    eng = nc.gpsimd if t in POOL_T else nc.vector
                    eng.tensor_scalar(
                        out=eqt[:, 0:W],
                        in0=cb[:, PAD + 128 * t + 131:PAD + L],
                        scalar1=code_ipart[:, t:t + 1],
                        scalar2=None, op0=AL.is_equal)
                    eq_sc[t] = eqt

                # count matmuls for all far tiles, scalar tiles first
                for t in sc_tiles + list(POOL_T) + list(DVE_T):
                    W = L - (128 * t + 131)
                    if W <= 0:
                        continue
                    eqt = eq_sc[t]
                    jlo = 128 * t + 131
                    for (a, b2) in _bank_chunks(jlo, L):
                        nc.tensor.matmul(
                            counts_ps[32 * r:32 * r + 1, a:b2], ones_bf,
                            eqt[:, a - jlo:b2 - jlo],
                            start=False, stop=True, skip_group_check=True)

            # =========== PHASE E: sumexp/pen (off the critical path) ======
            for r in range(R):
                sumexp = small.tile([128, NT], F32, name=f"sumexp{r}")
                for h, (b0, b1) in enumerate(CHUNKS[r]):
                    E0 = key_all[(r, h)][:, 0:(b1 - b0) * 128].bitcast(F32)
                    nc.vector.tensor_reduce(
                        out=sumexp[:, b0:b1],
                        in_=E0.rearrange("p (b v) -> p b v", v=128),
                        axis=mybir.AxisListType.X, op=AL.add)
                rcp = small.tile([128, NT], F32, name=f"rcp{r}")
                nc.vector.reciprocal(rcp[:], sumexp[:])

                # pp = maxexp / sumexp; maxexp = raw max bits as f32
                e1m = rk_all[r][:].bitcast(F32)
                pp = small.tile([128, NT], F32, name=f"pp{r}")
                nc.gpsimd.tensor_tensor(out=pp[:], in0=e1m, in1=rcp[:],
                                        op=AL.mult)

                # pen2 = pp + shift1(pp) + shift2(pp)
                ps_ppn = ps.tile([128, 2], F32, tag="ppn", name=f"ps_ppn{r}")
                nc.tensor.matmul(ps_ppn[:, 0:2], ishift, pp[:, 0:2],
                                 start=True, stop=True)
                pp_nxt = ps_ppn[:, 0:2]
                ph1 = small.tile([128, NT], F32, name=f"ph1{r}")
                ph2 = small.tile([128, NT], F32, name=f"ph2{r}")
                nc.gpsimd.tensor_copy(ph1[:, 0:NT - 1], pp[:, 1:NT])
                nc.gpsimd.tensor_copy(ph2[:, 0:NT - 2], pp[:, 2:NT])
                nc.vector.tensor_copy(ph1[:, NT - 1:NT], pp_nxt[:, 0:1])
                nc.vector.tensor_copy(ph2[:, NT - 2:NT - 1], pp_nxt[:, 0:1])
                nc.vector.tensor_copy(ph2[:, NT - 1:NT], pp_nxt[:, 1:2])
                pen2 = small.tile([128, NT], F32, name=f"pen2{r}")
                nc.gpsimd.tensor_tensor(out=pen2[:], in0=pp[:], in1=ph1[:],
                                        op=AL.add)
                nc.gpsimd.tensor_tensor(out=pen2[:], in0=pen2[:], in1=ph2[:],
                                        op=AL.add)

                # epilogue: counts (PSUM) -> SBUF -> spread -> dot with pen
                counts_sb = small.tile([1, T], F32, name=f"counts_sb{r}")
                counts_div = small.tile([128, NT], F32, name=f"counts_div{r}")
                junk16 = scrp.tile([128, NT], F32, tag=f"j16{r}", name=f"junk16{r}")
                nc.scalar.copy(counts_sb[0:1, 0:1024],
                               counts_ps[32 * r:32 * r + 1, 0:1024])
                nc.scalar.copy(counts_sb[0:1, 1024:T],
                               counts_ps[32 * r:32 * r + 1, 1024:T])
                nc.gpsimd.dma_start(counts_div[0:64, :], counts_sb[0:1, 0:1024])
                nc.gpsimd.dma_start(counts_div[64:128, :], counts_sb[0:1, 1024:T])
                nc.vector.scalar_tensor_tensor(
                    out=junk16[:], in0=counts_div[:],
                    scalar=1.0, in1=pen2[:],
                    op0=AL.mult, op1=AL.mult,
                    accum_out=s1c[:, r:r + 1])

            # ---------------- final scalar ----------------
            nc.tensor.matmul(ps_fin[:], ones_f32, s1c[:], start=True, stop=True)
            nc.vector.tensor_scalar(out=junkR[:], in0=ps_fin[:],
                                    scalar1=SCALE, scalar2=None,
                                    op0=AL.mult, op1=AL.add,
                                    accum_out=final_sb[:])
            nc.sync.dma_start(y_ext.ap()[:, :], final_sb[:])

    nc.compile()
    return nc


_NC_CACHE = None
_CONSTS = None


def _get_nc():
    global _NC_CACHE
    if _NC_CACHE is None:
        _NC_CACHE = build_nc()
    return _NC_CACHE


def _get_consts():
    global _CONSTS
    if _CONSTS is None:
        _CONSTS = _build_consts()
    return _CONSTS


def kernel(**inputs) -> np.ndarray:
    logits = np.ascontiguousarray(np.asarray(inputs["logits"], dtype=np.float32))
    assert logits.shape == (B, T, V), logits.shape
    nc = _get_nc()
    cst = _get_consts()
    in_maps = [
        {"logits": logits[i * R:(i + 1) * R].reshape(R * T, V), "consts": cst}
        for i in range(N_CORES)
    ]
    res = run_bass_kernel_spmd(nc, in_maps, core_ids=list(range(N_CORES)))
    total = np.float32(0.0)
    for i in range(N_CORES):
        total = total + res.results[i]["out"][0, 0]
    return np.asarray(total, dtype=np.float32)


# revision 29
# speedup vs baseline: 1.0648x; 1.0648x over previous
"""AntiPatternLoss Trainium2 kernel (8 NeuronCores, data-parallel over batch).

Reference computation (per batch row of logits [T=2048, V=128]):
  pred      = argmax_v(logits)                                    # [T]
  prob_pred = softmax(logits)[t, pred[t]] = maxexp / sum_v exp(l)
  pen[j]    = mean_{k<3} prob_pred[j+k]                           # [L], L = T-2
  eq[i,j]   = (trigram at i == trigram at j) and (j - i >= 3)
  loss      = REP_PEN * sum_j(count_j * pen_j) / (B*T)

Kernel strategy per core (2 rows):
  - Critical path = argmax -> trigram codes -> broadcast -> pairwise eq.
    Everything else (sumexp, pen, epilogue) is deferred off that path.
  - argmax per 128-token group via the bitcast key trick: E1 = exp(l/32)
    lies in [0.84, 1.19] so bits 7..23 of its f32 pattern are a 24-bit
    monotone key that survives the DVE's internal-f32 max exactly;
    the low 7 bits are replaced by (127 - v) so the max also carries the
    first-max index (ties break toward the first index like jnp.argmax).
  - sumexp via a TT-add tree over E0 = exp(l); pp = E1max_trunc^32/sumexp.
  - trigram code = p0*16384 + p1*128 + p2 (< 2^21, exact in fp32);
    pairwise match is ONE fp32 equality compare (2x DVE mode).
  - main O(L^2) loop: i on partitions; small-t far tiles on DVE
    tensor_scalar(is_equal), large-t tiles on ScalarE (Abs then Relu);
    the i-reduction runs on TensorE as ones-matmul accumulating
    counts[1, j] in PSUM.  Tiny [128,16] glue ops run on GpSimd.
  - partial diagonal blocks: per-tile eq + one staircase mask multiply
  - per-core partial loss scalars are summed on the host (gather step)
"""

import numpy as np

import concourse.mybir as mybir
from concourse import bacc, tile
from concourse.bass_utils import run_bass_kernel_spmd

F32 = mybir.dt.float32
BF16 = mybir.dt.bfloat16
F16 = mybir.dt.float16
I32 = mybir.dt.int32
AL = mybir.AluOpType
AF = mybir.ActivationFunctionType

N_CORES = 8
B, T, V = 16, 2048, 128
R = B // N_CORES          # rows per core = 2
NGRAM = 3
REP_PEN = 1.2
L = T - NGRAM + 1         # 2046 trigram start positions
NT = T // 128             # 16 i-tiles per row
PAD = 2                   # sentinel cols in front of codes in code_bcast
SENT_BC = -1.0            # j-side (code_bcast / code flat) sentinel
SENT_I = -3.0             # i-side (code_ipart) sentinel
SCALE = REP_PEN / (NGRAM * B * T)   # pen's /3 folded in
CB_W = PAD + T + 8        # code_bcast width
SPLIT_T = 6               # far tiles t < SPLIT_T on DVE, rest on ScalarE
NCH = 4
HALF = NT // NCH


# ---- host-packed constant table (one [128, CONST_W] f32 DMA) -----------
C_DIAG = 0
C_ID16 = 1024
C_ISH = 1032
C_ONEF = 1160
C_ONEB = 1161
C_SENTI = 1162
C_SENTB = 1163
C_WREVH = 1167
C_ZERO = 2191
CONST_W = 2447


def _pack_bf16(a32):
    bf = (np.ascontiguousarray(a32, np.float32).view(np.uint32) >> 16) \
        .astype(np.uint16)
    return np.ascontiguousarray(bf).view(np.float32)


def _build_consts() -> np.ndarray:
    c = np.zeros((128, CONST_W), dtype=np.float32)
    p = np.arange(128)[:, None]
    cc = np.arange(128)[None, :]
    stair = ((cc >= p) & (cc <= 126)).astype(np.float32)
    c[:, C_DIAG:C_DIAG + 1024] = _pack_bf16(np.tile(stair, (1, 16)))
    c[0:16, C_ID16:C_ID16 + 8] = _pack_bf16(np.eye(16, dtype=np.float32))
    c[:, C_ISH:C_ISH + 128] = (cc == (p - 1)).astype(np.float32)
    c[:, C_ONEF] = 1.0
    c[:, C_ONEB:C_ONEB + 1] = _pack_bf16(np.ones((128, 2), dtype=np.float32))
    c[0:2, C_SENTI] = SENT_I
    c[0, C_SENTB:C_SENTB + 4] = SENT_BC
    wrev = np.tile((127.0 - np.arange(128, dtype=np.float16))[None, :],
                   (128, 16))
    c[:, C_WREVH:C_WREVH + 1024] = np.ascontiguousarray(
        wrev.view(np.uint16)).view(np.float32)
    return np.ascontiguousarray(c)


def _bank_chunks(a, b):
    """Split [a, b) at 512-column PSUM bank boundaries."""
    out = []
    while a < b:
        nxt = min(b, (a // 512 + 1) * 512)
        out.append((a, nxt))
        a = nxt
    return out


def build_nc():
    nc = bacc.Bacc("TRN2", target_bir_lowering=False, debug=False,
                   num_devices=N_CORES)
    x_ext = nc.dram_tensor("logits", [R * T, V], F32, kind="ExternalInput")
    c_ext = nc.dram_tensor("consts", [128, CONST_W], F32, kind="ExternalInput")
    y_ext = nc.dram_tensor("out", [1, 1], F32, kind="ExternalOutput")

    with tile.TileContext(nc) as tc:
        with (
            tc.tile_pool(name="setup", bufs=1) as setup,
            tc.tile_pool(name="big", bufs=1) as big,
            tc.tile_pool(name="small", bufs=1) as small,
            tc.tile_pool(name="eqp", bufs=10) as eqp,
            tc.tile_pool(name="scr", bufs=1) as scrp,
            tc.tile_pool(name="ps", bufs=1, space="PSUM") as ps,
            tc.tile_pool(name="dram", bufs=1, space="DRAM") as dram,
        ):
            # ---------------- constants: one DMA on the gpsimd queue ------
            consts = setup.tile([128, CONST_W], F32)
            nc.gpsimd.dma_start(consts[:], c_ext.ap())
            diagmask = consts[:, C_DIAG:C_DIAG + 1024].bitcast(BF16)
            ident16 = consts[0:16, C_ID16:C_ID16 + 8].bitcast(BF16)
            ishift = consts[:, C_ISH:C_ISH + 128]
            ones_f32 = consts[:, C_ONEF:C_ONEF + 1]
            ones_bf = consts[:, C_ONEB:C_ONEB + 1].bitcast(BF16)[:, 0:1]
            sentI = consts[0:2, C_SENTI:C_SENTI + 1]
            sentBC = consts[0:1, C_SENTB:C_SENTB + 4]
            wrevh = consts[:, C_WREVH:C_WREVH + 1024].bitcast(F16)
            zeros512 = consts[:, C_ZERO:C_ZERO + 256].bitcast(BF16)

            counts_ps = ps.tile([32 * (R - 1) + 1, T], F32)
            ps_fin = ps.tile([1, R], F32)
            s1c = small.tile([128, R], F32)
            junkR = scrp.tile([1, R], F32)
            final_sb = scrp.tile([1, 1], F32)

            x = x_ext.ap()
            dmae = nc.sync

            # ====== per-row pipeline: row0 entirely first, then row1 ======
            # (ring tags let row1's DMA loads prefetch while row0 computes;
            #  emitting row0's whole chain first biases engine queues so
            #  row0's pair work starts ASAP and row1's preproc fills gaps)
            # row0's first chunk halved so its key chain starts earlier
            CHUNKS = {0: [(0, 2), (2, 4), (4, 8), (8, 12), (12, 16)],
                      1: [(0, 4), (4, 8), (8, 12), (12, 16)]}
            lgh_all = {}
            rk_all = {}

            for r in range(R):
                for h, (b0, b1) in enumerate(CHUNKS[r]):
                    lgh = big.tile([128, HALF * 128], F32, tag="lg", bufs=9,
                                   name=f"logits_sb{r}{h}")
                    lgh_all[(r, h)] = lgh[:, 0:(b1 - b0) * 128]
                    src = x[r * T:(r + 1) * T, :] \
                        .rearrange("(a b) v -> a (b v)", a=128)[:, b0 * 128 * V // 128:b1 * V]
                    dmae.dma_start(lgh[:, 0:(b1 - b0) * 128], src)
            for r in range(R):
                red_key = small.tile([128, NT], F16, name=f"rk_{r}")
                idxw = small.tile([128, NT], F16, name=f"idxw_{r}")
                rk_all[r] = red_key
                for h, (b0, b1) in enumerate(CHUNKS[r]):
                    w = (b1 - b0) * 128
                    nb = b1 - b0
                    lgh = lgh_all[(r, h)]
                    key_t = big.tile([128, HALF * 128], F16, tag="ky", bufs=3,
                                     name=f"key_{r}{h}")
                    key = key_t[:, 0:w]
                    # key = exp(l) in fp16: fp16 max = fp16 argmax value;
                    # ties resolve to the first index via the wrev trick
                    nc.scalar.activation(key, lgh, AF.Exp, bias=0.0, scale=1.0)
                    k3 = key.rearrange("p (b v) -> p b v", v=128)
                    bs = slice(b0, b1)
                    with nc.allow_low_precision(reason="fp16 max reduce"):
                        nc.vector.tensor_reduce(
                            out=red_key[:, bs], in_=k3,
                            axis=mybir.AxisListType.X, op=AL.max)
                    rkb = red_key[:, bs].rearrange("p (b o) -> p b o", o=1) \
                        .broadcast_to([128, nb, 128])
                    eqm_t = big.tile([128, HALF * 128], F16, tag="eqm",
                                     bufs=3, name=f"eqm_{r}{h}")
                    eqm = eqm_t[:, 0:w].rearrange("p (b v) -> p b v", v=128)
                    nc.vector.tensor_tensor(out=eqm, in0=k3, in1=rkb,
                                            op=AL.is_equal)
                    wb = wrevh[:, 0:w].rearrange("p (b v) -> p b v", v=128)
                    nc.vector.tensor_tensor(out=eqm, in0=eqm, in1=wb,
                                            op=AL.mult)
                    with nc.allow_low_precision(reason="fp16 max reduce"):
                        nc.vector.tensor_reduce(
                            out=idxw[:, bs], in_=eqm,
                            axis=mybir.AxisListType.X, op=AL.max)

                # decode pred as bf16 (values <= 127, exact)
                pred_bf = small.tile([128, NT], BF16, name=f"pred_bf{r}")
                nc.gpsimd.tensor_scalar(out=pred_bf[:], in0=idxw[:],
                                        scalar1=-1.0, scalar2=127.0,
                                        op0=AL.mult, op1=AL.add)

                # next-partition pred values via a tiny DMA shift
                pshift = small.tile([128, 2], BF16, name=f"pshift{r}")
                nc.gpsimd.memset(pshift[:], 0.0)  # junk fill (row 127 unused)
                dmae.dma_start(pshift[0:127, :], pred_bf[1:128, 0:2])

                # pred -> SBUF flat [16, 130] -> PE transposes
                flat16 = small.tile([16, 130], BF16, name=f"flat16_{r}")
                nc.vector.memset(flat16[:], 0.0)
                dmae.dma_start(flat16[0:16, 0:128], pred_bf[:])
                # wrap columns: flat[128q + 128 + c] = pred_bf[8(q+1), c]
                dmae.dma_start(
                    flat16[0:15, 128:130],
                    pred_bf[:].rearrange("(q e) b -> q e b", e=8)[1:16, 0, 0:2])
                # code_ipart[p, t] = code[128t + p] via 3 PE transposes
                tp_ps = ps.tile([128, 3 * NT], BF16, tag="tp", name=f"tp_ps{r}")
                for k in range(3):
                    nc.tensor.transpose(tp_ps[:, NT * k:NT * (k + 1)],
                                        flat16[0:16, k:k + 128], ident16)
                p0t = tp_ps[:, 0:NT]
                p1t = tp_ps[:, NT:2 * NT]
                p2t = tp_ps[:, 2 * NT:3 * NT]
                ipt_a = small.tile([128, NT], F32, name=f"ipt_a{r}")
                ipt_b = small.tile([128, NT], F32, name=f"ipt_b{r}")
                code_ipart = small.tile([128, NT], F32, name=f"code_ipart{r}")
                nc.vector.tensor_scalar(out=ipt_a[:], in0=p0t, scalar1=16384.0,
                                        scalar2=None, op0=AL.mult)
                nc.vector.scalar_tensor_tensor(out=ipt_b[:], in0=p1t, scalar=128.0,
                                               in1=ipt_a[:], op0=AL.mult, op1=AL.add)
                nc.vector.tensor_tensor(out=code_ipart[:], in0=ipt_b[:], in1=p2t,
                                        op=AL.add)
                dmae.dma_start(code_ipart[126:128, NT - 1:NT], sentI)
                negip = small.tile([128, NT], F32, name=f"negip{r}")
                nc.gpsimd.tensor_scalar(out=negip[:], in0=code_ipart[:],
                                        scalar1=-1.0, scalar2=None, op0=AL.mult)

                # trigram codes via shifted views of pred_bf + pshift patches
                tmp_a = small.tile([128, NT], F32, name=f"tmp_a{r}")
                tmp_b = small.tile([128, NT], F32, name=f"tmp_b{r}")
                code2 = small.tile([128, NT], F32, name=f"code2{r}")
                psh128 = small.tile([128, 2], F32, name=f"psh128_{r}")
                nc.gpsimd.tensor_scalar(out=tmp_a[:], in0=pred_bf[:],
                                        scalar1=16384.0, scalar2=None, op0=AL.mult)
                nc.gpsimd.tensor_scalar(out=psh128[:], in0=pshift[:],
                                        scalar1=128.0, scalar2=None, op0=AL.mult)
                nc.gpsimd.tensor_scalar(out=tmp_b[:, 0:NT - 1],
                                        in0=pred_bf[:, 1:NT], scalar1=128.0,
                                        scalar2=None, op0=AL.mult)
                nc.gpsimd.tensor_copy(tmp_b[:, NT - 1:NT], psh128[:, 0:1])
                nc.gpsimd.tensor_tensor(out=tmp_b[:], in0=tmp_b[:], in1=tmp_a[:],
                                        op=AL.add)
                nc.gpsimd.tensor_tensor(out=code2[:, 0:NT - 2], in0=tmp_b[:, 0:NT - 2],
                                        in1=pred_bf[:, 2:NT], op=AL.add)
                nc.gpsimd.tensor_tensor(out=code2[:, NT - 2:NT - 1],
                                        in0=tmp_b[:, NT - 2:NT - 1],
                                        in1=pshift[:, 0:1], op=AL.add)
                nc.gpsimd.tensor_tensor(out=code2[:, NT - 1:NT],
                                        in0=tmp_b[:, NT - 1:NT],
                                        in1=pshift[:, 1:2], op=AL.add)

                # -------- distribute codes ------------------------------
                # sentinel the last 2 code cells in SBUF, then stage via two
                # independent DRAM tiles (tail first) and broadcast back
                dmae.dma_start(code2[127:128, NT - 2:NT], sentBC[0:1, 0:2])
                cft = dram.tile([1, 1024], F32, name=f"cft{r}")
                cfh = dram.tile([1, 1024], F32, name=f"cfh{r}")
                cft128 = cft[:].rearrange("o (a b) -> (o a) b", a=64)
                cfh128 = cfh[:].rearrange("o (a b) -> (o a) b", a=64)
                dq = nc.gpsimd if r == 1 else dmae
                dq.dma_start(cft128[:, :], code2[64:128, :])
                dq.dma_start(cfh128[:, :], code2[0:64, :])

                cb = big.tile([128, CB_W], F32, tag=f"cb{r}", name=f"code_bcast{r}")
                nc.gpsimd.memset(cb[:, 0:PAD], SENT_BC)
                nc.gpsimd.memset(cb[:, PAD + T:CB_W], SENT_BC)
                dq.dma_start(cb[:, PAD + 1024:PAD + T],
                             cft[:].partition_broadcast(128))
                dq.dma_start(cb[:, PAD:PAD + 1024],
                             cfh[:].partition_broadcast(128))

                # -------- pairwise match counting ------------------------
                eqd = big.tile([128, NT * 128], BF16, tag=f"eqd{r}", name=f"eqd{r}")
                # PSUM zero-pass first (ready immediately -> runs before all
                # count matmuls; makes accumulation order-free)
                for (a, b2) in _bank_chunks(0, T):
                    nc.tensor.matmul(
                        counts_ps[32 * r:32 * r + 1, a:b2], ones_bf,
                        zeros512[:, 0:b2 - a],
                        start=True, stop=True, skip_group_check=True)
                for t in range(8, NT):
                    nc.vector.tensor_scalar(
                        out=eqd[:, 128 * t:128 * (t + 1)],
                        in0=cb[:, PAD + 128 * t + 3:PAD + 128 * t + 131],
                        scalar1=code_ipart[:, t:t + 1],
                        scalar2=None, op0=AL.is_equal)
                nc.vector.tensor_tensor(out=eqd[:, 1024:2048], in0=eqd[:, 1024:2048],
                                        in1=diagmask[:, 1024:2048], op=AL.mult)
                for (a, b2) in _bank_chunks(1027, L):
                    nc.tensor.matmul(
                        counts_ps[32 * r:32 * r + 1, a:b2], ones_bf,
                        eqd[:, a - 3:b2 - 3],
                        start=False, stop=True, skip_group_check=True)
                for t in range(0, 8):
                    nc.vector.tensor_scalar(
                        out=eqd[:, 128 * t:128 * (t + 1)],
                        in0=cb[:, PAD + 128 * t + 3:PAD + 128 * t + 131],
                        scalar1=code_ipart[:, t:t + 1],
                        scalar2=None, op0=AL.is_equal)
                nc.vector.tensor_tensor(out=eqd[:, 0:1024], in0=eqd[:, 0:1024],
                                        in1=diagmask[:, 0:1024], op=AL.mult)
                for (a, b2) in _bank_chunks(3, 1027):
                    nc.tensor.matmul(
                        counts_ps[32 * r:32 * r + 1, a:b2], ones_bf,
                        eqd[:, a - 3:b2 - 3],
                        start=False, stop=True, skip_group_check=True)
                for t in list(range(8, NT)) + [0, 1, 2, 3, 4, 5, 6, 7]:
                    W = L - (128 * t + 130)
                    if W <= 0:
                        continue
                    eqt = eqp.tile([128, 1920], BF16, tag="eqt", name=f"eqt{r}_{t}")
                    if t >= SPLIT_T:
                        # ScalarE path: |d| then relu(1 - |d|), exact on ints
                        at = eqp.tile([128, 1920], BF16, tag="at",
                                      name=f"at{r}_{t}")
                        nc.scalar.activation(at[:, 0:W],
                                             cb[:, PAD + 128 * t + 130:PAD + L],
                                             AF.Abs, bias=negip[:, t:t + 1],
                                             scale=1.0)
                        nc.scalar.activation(eqt[:, 0:W], at[:, 0:W],
                                             AF.Relu, bias=1.0, scale=-1.0)
                    else:
                        nc.vector.tensor_scalar(
                            out=eqt[:, 0:W],
                            in0=cb[:, PAD + 128 * t + 130:PAD + L],
                            scalar1=code_ipart[:, t:t + 1],
                            scalar2=None, op0=AL.is_equal)
                    jlo = 128 * t + 130
                    for (a, b2) in _bank_chunks(jlo, L):
                        nc.tensor.matmul(
                            counts_ps[32 * r:32 * r + 1, a:b2], ones_bf,
                            eqt[:, a - jlo:b2 - jlo],
                            start=False, stop=True, skip_group_check=True)

            # =========== PHASE E: sumexp/pen (off the critical path) ======
            for r in range(R):
                sumexp = small.tile([128, NT], F16, name=f"sumexp{r}")
                for h, (b0, b1) in enumerate(CHUNKS[r]):
                    lgh = lgh_all[(r, h)]
                    E0_t = big.tile([128, HALF * 128], F16, tag="e0", bufs=3,
                                    name=f"E0_{r}{h}")
                    E0 = E0_t[:, 0:(b1 - b0) * 128]
                    nc.scalar.activation(E0, lgh, AF.Exp, bias=0.0, scale=1.0)
                    with nc.allow_low_precision(reason="fp16 sumexp"):
                        nc.vector.tensor_reduce(
                            out=sumexp[:, b0:b1],
                            in_=E0.rearrange("p (b v) -> p b v", v=128),
                            axis=mybir.AxisListType.X, op=AL.add)
                sumf = small.tile([128, NT], F32, name=f"sumf{r}")
                nc.vector.tensor_copy(sumf[:], sumexp[:])
                rcp = small.tile([128, NT], F32, name=f"rcp{r}")
                nc.vector.reciprocal(rcp[:], sumf[:])

                # pp = maxexp / sumexp; maxexp = the fp16 max itself
                pp = small.tile([128, NT], F32, name=f"pp{r}")
                nc.gpsimd.tensor_tensor(out=pp[:], in0=rk_all[r][:],
                                        in1=rcp[:], op=AL.mult)

                # pen2 = pp + shift1(pp) + shift2(pp)
                ps_ppn = ps.tile([128, 2], F32, tag="ppn", name=f"ps_ppn{r}")
                nc.tensor.matmul(ps_ppn[:, 0:2], ishift, pp[:, 0:2],
                                 start=True, stop=True)
                pp_nxt = ps_ppn[:, 0:2]
                ph1 = small.tile([128, NT], F32, name=f"ph1{r}")
                ph2 = small.tile([128, NT], F32, name=f"ph2{r}")
                nc.gpsimd.tensor_copy(ph1[:, 0:NT - 1], pp[:, 1:NT])
                nc.gpsimd.tensor_copy(ph2[:, 0:NT - 2], pp[:, 2:NT])
                nc.vector.tensor_copy(ph1[:, NT - 1:NT], pp_nxt[:, 0:1])
                nc.vector.tensor_copy(ph2[:, NT - 2:NT - 1], pp_nxt[:, 0:1])
                nc.vector.tensor_copy(ph2[:, NT - 1:NT], pp_nxt[:, 1:2])
                pen2 = small.tile([128, NT], F32, name=f"pen2{r}")
                nc.gpsimd.tensor_tensor(out=pen2[:], in0=pp[:], in1=ph1[:],
                                        op=AL.add)
                nc.gpsimd.tensor_tensor(out=pen2[:], in0=pen2[:], in1=ph2[:],
                                        op=AL.add)

                # epilogue: counts (PSUM) -> SBUF -> spread -> dot with pen
                counts_sb = small.tile([1, T], F32, name=f"counts_sb{r}")
                counts_div = small.tile([128, NT], F32, name=f"counts_div{r}")
                junk16 = scrp.tile([128, NT], F32, tag=f"j16{r}", name=f"junk16{r}")
                nc.scalar.copy(counts_sb[0:1, 0:1024],
                               counts_ps[32 * r:32 * r + 1, 0:1024])
                nc.scalar.copy(counts_sb[0:1, 1024:T],
                               counts_ps[32 * r:32 * r + 1, 1024:T])
                dmae.dma_start(counts_div[0:64, :], counts_sb[0:1, 0:1024])
                dmae.dma_start(counts_div[64:128, :], counts_sb[0:1, 1024:T])
                nc.vector.scalar_tensor_tensor(
                    out=junk16[:], in0=counts_div[:],
                    scalar=1.0, in1=pen2[:],
                    op0=AL.mult, op1=AL.mult,
                    accum_out=s1c[:, r:r + 1])

            # ---------------- final scalar ----------------
            nc.tensor.matmul(ps_fin[:], ones_f32, s1c[:], start=True, stop=True)
            nc.vector.tensor_scalar(out=junkR[:], in0=ps_fin[:],
                                    scalar1=SCALE, scalar2=None,
                                    op0=AL.mult, op1=AL.add,
                                    accum_out=final_sb[:])
            nc.sync.dma_start(y_ext.ap()[:, :], final_sb[:])

    nc.compile()
    return nc


_NC_CACHE = None
_CONSTS = None


def _get_consts():
    global _CONSTS
    if _CONSTS is None:
        _CONSTS = _build_consts()
    return _CONSTS


def _get_nc():
    global _NC_CACHE
    if _NC_CACHE is None:
        _NC_CACHE = build_nc()
    return _NC_CACHE


def kernel(**inputs) -> np.ndarray:
    logits = np.ascontiguousarray(np.asarray(inputs["logits"], dtype=np.float32))
    assert logits.shape == (B, T, V), logits.shape
    nc = _get_nc()
    cst = _get_consts()
    in_maps = [
        {"logits": logits[i * R:(i + 1) * R].reshape(R * T, V), "consts": cst}
        for i in range(N_CORES)
    ]
    res = run_bass_kernel_spmd(nc, in_maps, core_ids=list(range(N_CORES)))
    total = np.float32(0.0)
    for i in range(N_CORES):
        total = total + res.results[i]["out"][0, 0]
    return np.asarray(total, dtype=np.float32)

